# revision 9
# baseline (speedup 1.0000x reference)
"""Cross-attention (B=4, C=256, H=W=64) Trainium2 Bass kernel.

Math (per batch b):
    t = target[b]  : [C, N]   (N = H*W = 4096)
    r = reference[b]
    q = Wq @ t + bq ; k = Wk @ r + bk ; v = Wv @ r + bv
    attn = softmax(q^T k / sqrt(C), axis=j)
    out = v @ attn^T + target

Sharding: 8 cores = 4 batches x 2 query-halves. Each core gets its
query slice of t (NQ=2048) and the full r for its batch, and produces
the unnormalized attention output o[c, i] = sum_j v[c,j] * exp(s_ij)
plus the softmax denominator se[i] = sum_j exp(s_ij). The host divides,
adds bv (exact: softmax rows sum to 1) and the residual.

Device layouts (all matmuls contract over the partition axis):
    q_sb[cb]  : [128, NQ]   bf16   (c_out block, query pixels)
    k_sb[cb]  : [128, N]    bf16
    v_sb      : [128, NJB*C] bf16  V^T per j-block: [j in block, c]
    scores    : S^T[j_block, i] so exp() runs on ACT right out of PSUM
    sum_j exp : ones-vector matmul on PE (accumulated with the AV pass)
"""

import os
import sys

import numpy as np

try:
    import concourse.bass as _probe  # noqa: F401
except ImportError:
    for _p in ("/opt/trn_rl_repo", "/root/.axon_site/_ro/trn_rl_repo"):
        if os.path.isdir(_p) and _p not in sys.path:
            sys.path.insert(0, _p)

import ml_dtypes

import concourse.bacc as bacc
import concourse.bass as bass  # noqa: F401
import concourse.mybir as mybir
import concourse.tile as tile
from concourse.bass_utils import run_bass_kernel_spmd

BF16 = mybir.dt.bfloat16
F32 = mybir.dt.float32
NPBF16 = ml_dtypes.bfloat16

B, C, H, W = 4, 256, 64, 64
N = H * W                 # 4096 key/value pixels per batch
NCORES = 8
NQ = (B * N) // NCORES    # 2048 query pixels per core
P = 128
CB = C // P               # 2 channel blocks
ICH = 512                 # query chunk (one PSUM bank of fp32)
NICH = NQ // ICH          # 4
NJB = N // P              # 32 key blocks
JG = 2                    # key blocks per exp batch (2 PSUM banks)
SCALE = float(C) ** -0.5

# Set by test harness: trace=True to collect an NTFF profile.
TRACE = False
LAST_RESULTS = None


def _build():
    nc = bacc.Bacc("TRN2", target_bir_lowering=False, debug=False,
                   num_devices=NCORES)

    t = nc.dram_tensor("t", [C, NQ], BF16, kind="ExternalInput")
    r = nc.dram_tensor("r", [C, N], BF16, kind="ExternalInput")
    wqT = nc.dram_tensor("wqT", [C, C], BF16, kind="ExternalInput")
    wkT = nc.dram_tensor("wkT", [C, C], BF16, kind="ExternalInput")
    wvT = nc.dram_tensor("wvT", [C, C], BF16, kind="ExternalInput")
    bq = nc.dram_tensor("bq", [C, 1], F32, kind="ExternalInput")
    bk = nc.dram_tensor("bk", [C, 1], F32, kind="ExternalInput")
    o = nc.dram_tensor("o", [C, NQ], F32, kind="ExternalOutput")
    se = nc.dram_tensor("se", [1, NQ], F32, kind="ExternalOutput")

    with tile.TileContext(nc) as tc:
        with (
            tc.tile_pool(name="persist", bufs=1) as persist,
            tc.tile_pool(name="epool", bufs=4) as epool,
            tc.tile_pool(name="outp", bufs=4) as outp,
            tc.tile_pool(name="ps_s", bufs=2, space="PSUM") as ps_s,
            tc.tile_pool(name="ps_av", bufs=2, space="PSUM") as ps_av,
            tc.tile_pool(name="ps_one", bufs=2, space="PSUM") as ps_one,
        ):
            # ---- load inputs -------------------------------------------------
            t_sb, r_sb, wq_sb, wk_sb, wv_sb, bq_sb, bk_sb = [], [], [], [], [], [], []
            for cc in range(CB):
                cs = slice(cc * P, (cc + 1) * P)
                tt = persist.tile([P, NQ], BF16, tag=f"t{cc}")
                nc.sync.dma_start(out=tt[:], in_=t[cs, :])
                t_sb.append(tt)
                rt = persist.tile([P, N], BF16, tag=f"r{cc}")
                nc.sync.dma_start(out=rt[:], in_=r[cs, :])
                r_sb.append(rt)
                for lst, src, nm in ((wq_sb, wqT, "wq"), (wk_sb, wkT, "wk"),
                                     (wv_sb, wvT, "wv")):
                    wtile = persist.tile([P, C], BF16, tag=f"{nm}{cc}")
                    nc.sync.dma_start(out=wtile[:], in_=src[cs, :])
                    lst.append(wtile)
                for lst, src, nm in ((bq_sb, bq, "bq"), (bk_sb, bk, "bk")):
                    btile = persist.tile([P, 1], F32, tag=f"{nm}{cc}")
                    nc.sync.dma_start(out=btile[:], in_=src[cs, :])
                    lst.append(btile)
            ones_sb = persist.tile([P, 1], BF16, tag="ones")
            nc.vector.memset(ones_sb[:], 1.0)

            # ---- projections -------------------------------------------------
            # q[o, i] = sum_c WqT[c, o] * t[c, i]  (+bq on the DVE copy)
            q_sb = [persist.tile([P, NQ], BF16, tag=f"q{ob}", name=f"q{ob}")
                    for ob in range(CB)]
            k_sb = [persist.tile([P, N], BF16, tag=f"k{ob}", name=f"k{ob}")
                    for ob in range(CB)]
            for ob in range(CB):
                os_ = slice(ob * P, (ob + 1) * P)
                for half in range(NQ // 1024):
                    qp = ps_s.tile([P, 1024], F32, tag="s")
                    for cc in range(CB):
                        for nch in range(2):
                            csl = slice(half * 1024 + nch * 512,
                                        half * 1024 + (nch + 1) * 512)
                            nc.tensor.matmul(
                                qp[:, nch * 512:(nch + 1) * 512],
                                lhsT=wq_sb[cc][:, os_],
                                rhs=t_sb[cc][:, csl],
                                start=(cc == 0), stop=(cc == CB - 1),
                            )
                    nc.vector.tensor_scalar_add(
                        q_sb[ob][:, half * 1024:(half + 1) * 1024], qp[:],
                        bq_sb[ob][:])
                for half in range(N // 1024):
                    kp = ps_s.tile([P, 1024], F32, tag="s")
                    for cc in range(CB):
                        for nch in range(2):
                            csl = slice(half * 1024 + nch * 512,
                                        half * 1024 + (nch + 1) * 512)
                            nc.tensor.matmul(
                                kp[:, nch * 512:(nch + 1) * 512],
                                lhsT=wk_sb[cc][:, os_],
                                rhs=r_sb[cc][:, csl],
                                start=(cc == 0), stop=(cc == CB - 1),
                            )
                    nc.vector.tensor_scalar_add(
                        k_sb[ob][:, half * 1024:(half + 1) * 1024], kp[:],
                        bk_sb[ob][:])

            # vT[j, c] = sum_c' r[c', j] * WvT[c', c]   (r is the stationary op)
            v_sb = persist.tile([P, NJB * C], BF16, tag="v")
            for jb in range(NJB):
                vp = ps_av.tile([P, C], F32, tag="av")
                for cc in range(CB):
                    nc.tensor.matmul(
                        vp[:],
                        lhsT=r_sb[cc][:, jb * P:(jb + 1) * P],
                        rhs=wv_sb[cc][:],
                        start=(cc == 0), stop=(cc == CB - 1),
                    )
                nc.vector.tensor_copy(out=v_sb[:, jb * C:(jb + 1) * C], in_=vp[:])

            # ---- attention ---------------------------------------------------
            for ic in range(NICH):
                isl = slice(ic * ICH, (ic + 1) * ICH)
                av = [ps_av.tile([P, ICH], F32, tag="av", name=f"av{ic}_{cb_}")
                      for cb_ in range(CB)]
                ones_ps = ps_one.tile([1, ICH], F32, tag="one")
                for jg in range(NJB // JG):
                    sps = ps_s.tile([P, JG * ICH], F32, tag="s")
                    for j2 in range(JG):
                        jb = jg * JG + j2
                        for cc in range(CB):
                            nc.tensor.matmul(
                                sps[:, j2 * ICH:(j2 + 1) * ICH],
                                lhsT=k_sb[cc][:, jb * P:(jb + 1) * P],
                                rhs=q_sb[cc][:, isl],
                                start=(cc == 0), stop=(cc == CB - 1),
                            )
                    et = epool.tile([P, JG * ICH], BF16, tag="e")
                    nc.scalar.activation(et[:], sps[:],
                                         mybir.ActivationFunctionType.Exp,
                                         scale=SCALE)
                    for j2 in range(JG):
                        jb = jg * JG + j2
                        e_sl = et[:, j2 * ICH:(j2 + 1) * ICH]
                        for cb in range(CB):
                            nc.tensor.matmul(
                                av[cb][:],
                                lhsT=v_sb[:, jb * C + cb * P:jb * C + (cb + 1) * P],
                                rhs=e_sl,
                                start=(jb == 0), stop=(jb == NJB - 1),
                            )
                        nc.tensor.matmul(
                            ones_ps[:], lhsT=ones_sb[:], rhs=e_sl,
                            start=(jb == 0), stop=(jb == NJB - 1),
                        )
                for cb in range(CB):
                    ot = outp.tile([P, ICH], F32, tag="o")
                    nc.vector.tensor_copy(out=ot[:], in_=av[cb][:])
                    nc.sync.dma_start(out=o[cb * P:(cb + 1) * P, isl],
                                      in_=ot[:])
                st = outp.tile([1, ICH], F32, tag="se")
                nc.vector.tensor_copy(out=st[:], in_=ones_ps[:])
                nc.sync.dma_start(out=se[0:1, isl], in_=st[:])

    nc.finalize()
    return nc


_NC_CACHE = None


def kernel(target, reference, Wq, bq, Wk, bk, Wv, bv):
    global _NC_CACHE, LAST_RESULTS
    target = np.asarray(target, np.float32)
    reference = np.asarray(reference, np.float32)
    Wq, Wk, Wv = (np.asarray(w, np.float32) for w in (Wq, Wk, Wv))
    bq, bk, bv = (np.asarray(b_, np.float32) for b_ in (bq, bk, bv))

    if _NC_CACHE is None:
        _NC_CACHE = _build()
    nc = _NC_CACHE

    t_full = target.reshape(B, C, N)
    r_full = reference.reshape(B, C, N)
    w_common = {
        "wqT": np.ascontiguousarray(Wq.T).astype(NPBF16),
        "wkT": np.ascontiguousarray(Wk.T).astype(NPBF16),
        "wvT": np.ascontiguousarray(Wv.T).astype(NPBF16),
        "bq": bq.reshape(C, 1),
        "bk": bk.reshape(C, 1),
    }
    in_maps = []
    for cid in range(NCORES):
        b_, h_ = cid // 2, cid % 2
        in_maps.append({
            "t": np.ascontiguousarray(
                t_full[b_][:, h_ * NQ:(h_ + 1) * NQ]).astype(NPBF16),
            "r": r_full[b_].astype(NPBF16),
            **w_common,
        })

    res = run_bass_kernel_spmd(
        nc, in_maps, core_ids=list(range(NCORES)), trace=TRACE,
    )
    LAST_RESULTS = res

    out = np.empty((B, C, N), np.float32)
    for cid in range(NCORES):
        b_, h_ = cid // 2, cid % 2
        o = res.results[cid]["o"].astype(np.float64)
        s = res.results[cid]["se"].astype(np.float64).reshape(NQ)
        sl = slice(h_ * NQ, (h_ + 1) * NQ)
        out[b_][:, sl] = (o / s[None, :] + bv.astype(np.float64)[:, None]
                          + t_full[b_][:, sl])
    return out.reshape(B, C, H, W)


# revision 10
# speedup vs baseline: 1.3486x; 1.3486x over previous
"""Cross-attention (B=4, C=256, H=W=64) Trainium2 Bass kernel.

Math (per batch b), with t = target[b] : [C, N], r = reference[b], N = H*W:
    q = Wq t + bq ; k = Wk r + bk ; v = Wv r + bv
    attn = softmax(q^T k / sqrt(C), axis=j)
    out = v attn^T + t

Sharding: 8 cores = 4 batches x 2 query-halves. Each core handles its
query slice of t (NQ = 2048) and the full r of its batch.

Algebraic folds (all exact):
  * scores: q_i . k_j = t_i^T (Wq^T Wk) r_j + bq.(Wk r_j) + (Wq t_i).bk + bq.bk
    The last two terms are per-query constants -> cancel in softmax.
    So with M = Wq^T Wk and g = Wk^T bq:  s[i,j] ~ r_j . u_i  where
    u = M^T t + g.  M, g are precomputed on the host.
  * bv: softmax rows sum to 1, so v -> v + bv just adds bv to the output;
    the host adds it.
  * normalization: the device returns o[c,i] = sum_j v[c,j] exp(s_ij)
    and the bf16 exp-matrix E; the host divides by colsum(E) (the exact
    denominator the AV matmul consumed) and adds the residual.

Device layouts (matmuls contract over the partition axis):
    u_sb[bb] : [128, NQ]  bf16   scores rhs
    r_sb[cc] : [128, N]   bf16   scores + v-projection stationary operand
    v_sb     : [128, NJB*C] bf16  V^T per key block: [j in block, c]
    scores   : S^T[j_blk, (ic2, i)] in a [128, 1024] PSUM tile; one exp
               (ACT) per key block covering a PAIR of query chunks, so
               each stationary tile serves 2 back-to-back matmuls.
"""

import os
import sys

import numpy as np

try:
    import concourse.bass as _probe  # noqa: F401
except ImportError:
    for _p in ("/opt/trn_rl_repo", "/root/.axon_site/_ro/trn_rl_repo"):
        if os.path.isdir(_p) and _p not in sys.path:
            sys.path.insert(0, _p)

import ml_dtypes

import concourse.bacc as bacc
import concourse.mybir as mybir
import concourse.tile as tile
from concourse.bass_utils import run_bass_kernel_spmd

BF16 = mybir.dt.bfloat16
F32 = mybir.dt.float32
NPBF16 = ml_dtypes.bfloat16

B, C, H, W = 4, 256, 64, 64
N = H * W                 # 4096 key/value pixels per batch
NCORES = 8
NQ = (B * N) // NCORES    # 2048 query pixels per core
P = 128
CB = C // P               # 2 channel blocks
ICH = 512                 # query chunk (one PSUM bank of fp32)
NICH = NQ // ICH          # 4
NJB = N // P              # 32 key blocks
SCALE = float(C) ** -0.5

# Set by test harness: trace=True to collect an NTFF profile.
TRACE = False
LAST_RESULTS = None


def _build():
    nc = bacc.Bacc("TRN2", target_bir_lowering=False, debug=False,
                   num_devices=NCORES)

    t = nc.dram_tensor("t", [C, NQ], BF16, kind="ExternalInput")
    r = nc.dram_tensor("r", [C, N], BF16, kind="ExternalInput")
    m = nc.dram_tensor("m", [C, C], BF16, kind="ExternalInput")
    wvT = nc.dram_tensor("wvT", [C, C], BF16, kind="ExternalInput")
    g = nc.dram_tensor("g", [C, 1], F32, kind="ExternalInput")
    o = nc.dram_tensor("o", [C, NQ], F32, kind="ExternalOutput")
    e_out = nc.dram_tensor("e_out", [N, NQ], BF16, kind="ExternalOutput")

    with tile.TileContext(nc) as tc:
        with (
            tc.tile_pool(name="persist", bufs=1) as persist,
            tc.tile_pool(name="epool", bufs=4) as epool,
            tc.tile_pool(name="outp", bufs=4) as outp,
            tc.tile_pool(name="ps_s", bufs=2, space="PSUM") as ps_s,
            tc.tile_pool(name="ps_av", bufs=4, space="PSUM") as ps_av,
        ):
            # ---- load inputs (r/t in column chunks so compute starts early)
            t_sb, r_sb, m_sb, wv_sb, g_sb = [], [], [], [], []
            for cc in range(CB):
                cs = slice(cc * P, (cc + 1) * P)
                rt = persist.tile([P, N], BF16, tag=f"r{cc}")
                for ch in range(4):
                    nc.sync.dma_start(out=rt[:, ch * 1024:(ch + 1) * 1024],
                                      in_=r[cs, ch * 1024:(ch + 1) * 1024])
                r_sb.append(rt)
                tt = persist.tile([P, NQ], BF16, tag=f"t{cc}")
                for ch in range(2):
                    nc.sync.dma_start(out=tt[:, ch * 1024:(ch + 1) * 1024],
                                      in_=t[cs, ch * 1024:(ch + 1) * 1024])
                t_sb.append(tt)
                for lst, src, nm in ((m_sb, m, "m"), (wv_sb, wvT, "wv")):
                    wtile = persist.tile([P, C], BF16, tag=f"{nm}{cc}")
                    nc.sync.dma_start(out=wtile[:], in_=src[cs, :])
                    lst.append(wtile)
                gt = persist.tile([P, 1], F32, tag=f"g{cc}")
                nc.sync.dma_start(out=gt[:], in_=g[cs, :])
                g_sb.append(gt)

            # ---- projections ------------------------------------------------
            # u[b, i] = sum_a m[a, b] t[a, i]  (+g on the DVE copy)
            u_sb = [persist.tile([P, NQ], BF16, tag=f"u{bb}", name=f"u{bb}")
                    for bb in range(CB)]
            for bb in range(CB):
                bs = slice(bb * P, (bb + 1) * P)
                for half in range(NQ // 1024):
                    up = ps_s.tile([P, 1024], F32, tag="s")
                    for ac in range(CB):
                        for nch in range(2):
                            csl = slice(half * 1024 + nch * 512,
                                        half * 1024 + (nch + 1) * 512)
                            nc.tensor.matmul(
                                up[:, nch * 512:(nch + 1) * 512],
                                lhsT=m_sb[ac][:, bs],
                                rhs=t_sb[ac][:, csl],
                                start=(ac == 0), stop=(ac == CB - 1),
                            )
                    nc.vector.tensor_scalar_add(
                        u_sb[bb][:, half * 1024:(half + 1) * 1024], up[:],
                        g_sb[bb][:])

            # vT[j, c] = sum_c' r[c', j] wvT[c', c]  (r is the stationary op);
            # two key blocks share one PSUM tile and one DVE copy.
            v_sb = persist.tile([P, NJB * C], BF16, tag="v")
            for jp in range(NJB // 2):
                vp = ps_av.tile([P, 2 * C], F32, tag="av")
                for j2 in range(2):
                    jb = 2 * jp + j2
                    for cc in range(CB):
                        nc.tensor.matmul(
                            vp[:, j2 * C:(j2 + 1) * C],
                            lhsT=r_sb[cc][:, jb * P:(jb + 1) * P],
                            rhs=wv_sb[cc][:],
                            start=(cc == 0), stop=(cc == CB - 1),
                        )
                nc.vector.tensor_copy(out=v_sb[:, jp * 2 * C:(jp + 1) * 2 * C],
                                      in_=vp[:])

            # ---- attention: pairs of query chunks ---------------------------
            for icp in range(NICH // 2):
                av = [ps_av.tile([P, ICH], F32, tag="av", name=f"av{icp}_{k}")
                      for k in range(2 * CB)]  # index = cb * 2 + ic2
                for jb in range(NJB):
                    sps = ps_s.tile([P, 2 * ICH], F32, tag="s")
                    for cc in range(CB):
                        for ic2 in range(2):
                            isl = slice((2 * icp + ic2) * ICH,
                                        (2 * icp + ic2 + 1) * ICH)
                            nc.tensor.matmul(
                                sps[:, ic2 * ICH:(ic2 + 1) * ICH],
                                lhsT=r_sb[cc][:, jb * P:(jb + 1) * P],
                                rhs=u_sb[cc][:, isl],
                                start=(cc == 0), stop=(cc == CB - 1),
                            )
                    et = epool.tile([P, 2 * ICH], BF16, tag="e")
                    nc.scalar.activation(et[:], sps[:],
                                         mybir.ActivationFunctionType.Exp,
                                         scale=SCALE)
                    nc.sync.dma_start(
                        out=e_out[jb * P:(jb + 1) * P,
                                  icp * 2 * ICH:(icp + 1) * 2 * ICH],
                        in_=et[:])
                    for cb in range(CB):
                        for ic2 in range(2):
                            nc.tensor.matmul(
                                av[cb * 2 + ic2][:],
                                lhsT=v_sb[:, jb * C + cb * P:
                                          jb * C + (cb + 1) * P],
                                rhs=et[:, ic2 * ICH:(ic2 + 1) * ICH],
                                start=(jb == 0), stop=(jb == NJB - 1),
                            )
                for cb in range(CB):
                    for ic2 in range(2):
                        isl = slice((2 * icp + ic2) * ICH,
                                    (2 * icp + ic2 + 1) * ICH)
                        ot = outp.tile([P, ICH], F32, tag="o")
                        nc.vector.tensor_copy(out=ot[:], in_=av[cb * 2 + ic2][:])
                        nc.sync.dma_start(out=o[cb * P:(cb + 1) * P, isl],
                                          in_=ot[:])

    nc.finalize()
    return nc


_NC_CACHE = None


def kernel(target, reference, Wq, bq, Wk, bk, Wv, bv):
    global _NC_CACHE, LAST_RESULTS
    target = np.asarray(target, np.float32)
    reference = np.asarray(reference, np.float32)
    Wq, Wk, Wv = (np.asarray(w, np.float32) for w in (Wq, Wk, Wv))
    bq, bk, bv = (np.asarray(b_, np.float32) for b_ in (bq, bk, bv))

    if _NC_CACHE is None:
        _NC_CACHE = _build()
    nc = _NC_CACHE

    t_full = target.reshape(B, C, N)
    r_full = reference.reshape(B, C, N)
    m_mat = (Wq.T @ Wk).astype(NPBF16)           # scores fold: M = Wq^T Wk
    g_vec = (Wk.T @ bq).reshape(C, 1)            # bq fold (bk cancels exactly)
    w_common = {
        "m": m_mat,
        "wvT": np.ascontiguousarray(Wv.T).astype(NPBF16),
        "g": g_vec,
    }
    in_maps = []
    for cid in range(NCORES):
        b_, h_ = cid // 2, cid % 2
        in_maps.append({
            "t": np.ascontiguousarray(
                t_full[b_][:, h_ * NQ:(h_ + 1) * NQ]).astype(NPBF16),
            "r": r_full[b_].astype(NPBF16),
            **w_common,
        })

    res = run_bass_kernel_spmd(
        nc, in_maps, core_ids=list(range(NCORES)), trace=TRACE,
    )
    LAST_RESULTS = res

    out = np.empty((B, C, N), np.float32)
    for cid in range(NCORES):
        b_, h_ = cid // 2, cid % 2
        o = res.results[cid]["o"].astype(np.float64)
        den = res.results[cid]["e_out"].astype(np.float32).sum(
            axis=0, dtype=np.float64)
        sl = slice(h_ * NQ, (h_ + 1) * NQ)
        out[b_][:, sl] = (o / den[None, :] + bv.astype(np.float64)[:, None]
                          + t_full[b_][:, sl])
    return out.reshape(B, C, H, W)


# revision 12
# speedup vs baseline: 1.4794x; 1.0970x over previous
"""Cross-attention (B=4, C=256, H=W=64) Trainium2 Bass kernel.

Math (per batch b), with t = target[b] : [C, N], r = reference[b], N = H*W:
    q = Wq t + bq ; k = Wk r + bk ; v = Wv r + bv
    attn = softmax(q^T k / sqrt(C), axis=j)
    out = v attn^T + t

Sharding: 8 cores = 4 batches x 2 query-halves. Each core handles its
query slice of t (NQ = 2048) and the full r of its batch.

Algebraic folds (all exact):
  * scores: q_i . k_j = t_i^T (Wq^T Wk) r_j + bq.(Wk r_j) + (Wq t_i).bk + bq.bk
    The last two terms are per-query constants -> cancel in softmax.
    So with M = Wq^T Wk and g = Wk^T bq:  s[i,j] ~ r_j . u_i  where
    u = M^T t + g.  M, g are precomputed on the host.
  * bv: softmax rows sum to 1, so v -> v + bv just adds bv to the output;
    the host adds it.
  * normalization: the device returns o[c,i] = sum_j v[c,j] exp(s_ij)
    and the bf16 exp-matrix E; the host divides by colsum(E) (the exact
    denominator the AV matmul consumed) and adds the residual.

Device layouts (matmuls contract over the partition axis):
    u_sb[bb] : [128, NQ]  bf16   scores rhs
    r_sb[cc] : [128, N]   bf16   scores + v-projection stationary operand
    v_sb     : [128, NJB*C] bf16  V^T per key block: [j in block, c]
    scores   : S^T[j_blk, (ic2, i)] in a [128, 1024] PSUM tile; one exp
               (ACT) per key block covering a PAIR of query chunks, so
               each stationary tile serves 2 back-to-back matmuls.
"""

import os
import sys

import numpy as np

try:
    import concourse.bass as _probe  # noqa: F401
except ImportError:
    for _p in ("/opt/trn_rl_repo", "/root/.axon_site/_ro/trn_rl_repo"):
        if os.path.isdir(_p) and _p not in sys.path:
            sys.path.insert(0, _p)

import ml_dtypes

import concourse.bacc as bacc
import concourse.mybir as mybir
import concourse.tile as tile
from concourse.bass_utils import run_bass_kernel_spmd

BF16 = mybir.dt.bfloat16
F32 = mybir.dt.float32
NPBF16 = ml_dtypes.bfloat16

B, C, H, W = 4, 256, 64, 64
N = H * W                 # 4096 key/value pixels per batch
NCORES = 8
NQ = (B * N) // NCORES    # 2048 query pixels per core
P = 128
CB = C // P               # 2 channel blocks
ICH = 512                 # query chunk (one PSUM bank of fp32)
NICH = NQ // ICH          # 4
NJB = N // P              # 32 key blocks
SCALE = float(C) ** -0.5

# Set by test harness: trace=True to collect an NTFF profile.
TRACE = False
LAST_RESULTS = None


def _build():
    nc = bacc.Bacc("TRN2", target_bir_lowering=False, debug=False,
                   num_devices=NCORES)

    t = nc.dram_tensor("t", [C, NQ], BF16, kind="ExternalInput")
    r = nc.dram_tensor("r", [C, N], BF16, kind="ExternalInput")
    m = nc.dram_tensor("m", [C, C], BF16, kind="ExternalInput")
    wvT = nc.dram_tensor("wvT", [C, C], BF16, kind="ExternalInput")
    g = nc.dram_tensor("g", [C, 1], F32, kind="ExternalInput")
    o = nc.dram_tensor("o", [C, NQ], F32, kind="ExternalOutput")
    e_out = nc.dram_tensor("e_out", [N, NQ], BF16, kind="ExternalOutput")

    with tile.TileContext(nc) as tc:
        with (
            tc.tile_pool(name="persist", bufs=1) as persist,
            tc.tile_pool(name="epool", bufs=4) as epool,
            tc.tile_pool(name="outp", bufs=4) as outp,
            tc.tile_pool(name="ps_s", bufs=2, space="PSUM") as ps_s,
            tc.tile_pool(name="ps_av", bufs=4, space="PSUM") as ps_av,
        ):
            # ---- load inputs: small constants first, then r/t interleaved in
            # column chunks so the first projection matmuls start early.
            t_sb, r_sb, m_sb, wv_sb, g_sb = [], [], [], [], []
            for cc in range(CB):
                cs = slice(cc * P, (cc + 1) * P)
                for lst, src, nm in ((m_sb, m, "m"), (wv_sb, wvT, "wv")):
                    wtile = persist.tile([P, C], BF16, tag=f"{nm}{cc}")
                    nc.sync.dma_start(out=wtile[:], in_=src[cs, :])
                    lst.append(wtile)
                gt = persist.tile([P, 1], F32, tag=f"g{cc}")
                nc.sync.dma_start(out=gt[:], in_=g[cs, :])
                g_sb.append(gt)
                r_sb.append(persist.tile([P, N], BF16, tag=f"r{cc}",
                                         name=f"r{cc}"))
                t_sb.append(persist.tile([P, NQ], BF16, tag=f"t{cc}",
                                         name=f"t{cc}"))
            for ch in range(4):
                for cc in range(CB):
                    cs = slice(cc * P, (cc + 1) * P)
                    nc.sync.dma_start(
                        out=r_sb[cc][:, ch * 1024:(ch + 1) * 1024],
                        in_=r[cs, ch * 1024:(ch + 1) * 1024])
                    if ch < 2:
                        nc.sync.dma_start(
                            out=t_sb[cc][:, ch * 1024:(ch + 1) * 1024],
                            in_=t[cs, ch * 1024:(ch + 1) * 1024])

            # ---- projections ------------------------------------------------
            # u[b, i] = sum_a m[a, b] t[a, i]  (+g on the DVE copy)
            u_sb = [persist.tile([P, NQ], BF16, tag=f"u{bb}", name=f"u{bb}")
                    for bb in range(CB)]
            for bb in range(CB):
                bs = slice(bb * P, (bb + 1) * P)
                for half in range(NQ // 1024):
                    up = ps_s.tile([P, 1024], F32, tag="s")
                    for ac in range(CB):
                        for nch in range(2):
                            csl = slice(half * 1024 + nch * 512,
                                        half * 1024 + (nch + 1) * 512)
                            nc.tensor.matmul(
                                up[:, nch * 512:(nch + 1) * 512],
                                lhsT=m_sb[ac][:, bs],
                                rhs=t_sb[ac][:, csl],
                                start=(ac == 0), stop=(ac == CB - 1),
                            )
                    nc.vector.tensor_scalar_add(
                        u_sb[bb][:, half * 1024:(half + 1) * 1024], up[:],
                        g_sb[bb][:])

            # vT[j, c] = sum_c' r[c', j] wvT[c', c]  (r is the stationary op);
            # two key blocks share one PSUM tile and one DVE copy.
            v_sb = persist.tile([P, NJB * C], BF16, tag="v")
            for jp in range(NJB // 2):
                vp = ps_av.tile([P, 2 * C], F32, tag="av")
                for j2 in range(2):
                    jb = 2 * jp + j2
                    for cc in range(CB):
                        nc.tensor.matmul(
                            vp[:, j2 * C:(j2 + 1) * C],
                            lhsT=r_sb[cc][:, jb * P:(jb + 1) * P],
                            rhs=wv_sb[cc][:],
                            start=(cc == 0), stop=(cc == CB - 1),
                        )
                nc.vector.tensor_copy(out=v_sb[:, jp * 2 * C:(jp + 1) * 2 * C],
                                      in_=vp[:])

            # ---- attention: pairs of query chunks ---------------------------
            # The AV pass runs one key block behind the score pass, so each
            # exp (ACT, ~1.1us) hides under the next block's score matmuls.
            for icp in range(NICH // 2):
                av = [ps_av.tile([P, ICH], F32, tag="av", name=f"av{icp}_{k}")
                      for k in range(2 * CB)]  # index = cb * 2 + ic2
                ets = {}

                def emit_scores(jb, icp=icp, ets=ets):
                    sps = ps_s.tile([P, 2 * ICH], F32, tag="s", name="sps")
                    for cc in range(CB):
                        for ic2 in range(2):
                            isl = slice((2 * icp + ic2) * ICH,
                                        (2 * icp + ic2 + 1) * ICH)
                            nc.tensor.matmul(
                                sps[:, ic2 * ICH:(ic2 + 1) * ICH],
                                lhsT=r_sb[cc][:, jb * P:(jb + 1) * P],
                                rhs=u_sb[cc][:, isl],
                                start=(cc == 0), stop=(cc == CB - 1),
                            )
                    et = epool.tile([P, 2 * ICH], BF16, tag="e", name="et")
                    nc.scalar.activation(et[:], sps[:],
                                         mybir.ActivationFunctionType.Exp,
                                         scale=SCALE)
                    nc.sync.dma_start(
                        out=e_out[jb * P:(jb + 1) * P,
                                  icp * 2 * ICH:(icp + 1) * 2 * ICH],
                        in_=et[:])
                    ets[jb] = et

                def emit_av(jb, av=av, ets=ets):
                    et = ets.pop(jb)
                    for cb in range(CB):
                        for ic2 in range(2):
                            nc.tensor.matmul(
                                av[cb * 2 + ic2][:],
                                lhsT=v_sb[:, jb * C + cb * P:
                                          jb * C + (cb + 1) * P],
                                rhs=et[:, ic2 * ICH:(ic2 + 1) * ICH],
                                start=(jb == 0), stop=(jb == NJB - 1),
                            )

                emit_scores(0)
                for jb in range(1, NJB):
                    emit_scores(jb)
                    emit_av(jb - 1)
                emit_av(NJB - 1)

                for cb in range(CB):
                    for ic2 in range(2):
                        isl = slice((2 * icp + ic2) * ICH,
                                    (2 * icp + ic2 + 1) * ICH)
                        ot = outp.tile([P, ICH], F32, tag="o")
                        nc.vector.tensor_copy(out=ot[:], in_=av[cb * 2 + ic2][:])
                        nc.sync.dma_start(out=o[cb * P:(cb + 1) * P, isl],
                                          in_=ot[:])

    nc.finalize()
    return nc


_NC_CACHE = None


def kernel(target, reference, Wq, bq, Wk, bk, Wv, bv):
    global _NC_CACHE, LAST_RESULTS
    target = np.asarray(target, np.float32)
    reference = np.asarray(reference, np.float32)
    Wq, Wk, Wv = (np.asarray(w, np.float32) for w in (Wq, Wk, Wv))
    bq, bk, bv = (np.asarray(b_, np.float32) for b_ in (bq, bk, bv))

    if _NC_CACHE is None:
        _NC_CACHE = _build()
    nc = _NC_CACHE

    t_full = target.reshape(B, C, N)
    r_full = reference.reshape(B, C, N)
    m_mat = (Wq.T @ Wk).astype(NPBF16)           # scores fold: M = Wq^T Wk
    g_vec = (Wk.T @ bq).reshape(C, 1)            # bq fold (bk cancels exactly)
    w_common = {
        "m": m_mat,
        "wvT": np.ascontiguousarray(Wv.T).astype(NPBF16),
        "g": g_vec,
    }
    in_maps = []
    for cid in range(NCORES):
        b_, h_ = cid // 2, cid % 2
        in_maps.append({
            "t": np.ascontiguousarray(
                t_full[b_][:, h_ * NQ:(h_ + 1) * NQ]).astype(NPBF16),
            "r": r_full[b_].astype(NPBF16),
            **w_common,
        })

    res = run_bass_kernel_spmd(
        nc, in_maps, core_ids=list(range(NCORES)), trace=TRACE,
    )
    LAST_RESULTS = res

    out = np.empty((B, C, N), np.float32)
    for cid in range(NCORES):
        b_, h_ = cid // 2, cid % 2
        o = res.results[cid]["o"].astype(np.float64)
        den = res.results[cid]["e_out"].astype(np.float32).sum(
            axis=0, dtype=np.float64)
        sl = slice(h_ * NQ, (h_ + 1) * NQ)
        out[b_][:, sl] = (o / den[None, :] + bv.astype(np.float64)[:, None]
                          + t_full[b_][:, sl])
    return out.reshape(B, C, H, W)


# revision 13
# speedup vs baseline: 1.5414x; 1.0419x over previous
"""Cross-attention (B=4, C=256, H=W=64) Trainium2 Bass kernel.

Math (per batch b), with t = target[b] : [C, N], r = reference[b], N = H*W:
    q = Wq t + bq ; k = Wk r + bk ; v = Wv r + bv
    attn = softmax(q^T k / sqrt(C), axis=j)
    out = v attn^T + t

Sharding: 8 cores = 4 batches x 2 query-halves. Each core handles its
query slice of t (NQ = 2048) and the full r of its batch.

Algebraic folds (all exact):
  * scores: q_i . k_j = t_i^T (Wq^T Wk) r_j + bq.(Wk r_j) + (Wq t_i).bk + bq.bk
    The last two terms are per-query constants -> cancel in softmax.
    So with M = Wq^T Wk and g = Wk^T bq:  s[i,j] ~ r_j . u_i  where
    u = M^T t + g.  M, g are precomputed on the host.
  * bv: softmax rows sum to 1, so v -> v + bv just adds bv to the output;
    the host adds it.
  * normalization: the device returns o[c,i] = sum_j v[c,j] exp(s_ij)
    and the bf16 exp-matrix E; the host divides by colsum(E) (the exact
    denominator the AV matmul consumed) and adds the residual.

Device layouts (matmuls contract over the partition axis):
    u_sb[bb] : [128, NQ]  bf16   scores rhs
    r_sb[cc][ch] : [128, 1024] bf16 x4   scores + v-proj stationary operand
                  (chunked tiles so compute starts as soon as data lands)
    v_sb     : [128, NJB*C] bf16  V^T per key block: [j in block, c]
    scores   : S^T[j_blk, (ic2, i)] in a [128, 1024] PSUM tile; one exp
               (ACT) per key block covering a PAIR of query chunks, so
               each stationary tile serves 2 back-to-back matmuls; the
               AV pass runs one key block behind so exp latency hides.
"""

import os
import sys

import numpy as np

try:
    import concourse.bass as _probe  # noqa: F401
except ImportError:
    for _p in ("/opt/trn_rl_repo", "/root/.axon_site/_ro/trn_rl_repo"):
        if os.path.isdir(_p) and _p not in sys.path:
            sys.path.insert(0, _p)

import ml_dtypes

import concourse.bacc as bacc
import concourse.mybir as mybir
import concourse.tile as tile
from concourse.bass_utils import run_bass_kernel_spmd

BF16 = mybir.dt.bfloat16
F32 = mybir.dt.float32
NPBF16 = ml_dtypes.bfloat16

B, C, H, W = 4, 256, 64, 64
N = H * W                 # 4096 key/value pixels per batch
NCORES = 8
NQ = (B * N) // NCORES    # 2048 query pixels per core
P = 128
CB = C // P               # 2 channel blocks
ICH = 512                 # query chunk (one PSUM bank of fp32)
NICH = NQ // ICH          # 4
NJB = N // P              # 32 key blocks
RCH = 1024                # r/t chunk width (per-chunk SBUF tiles)
SCALE = float(C) ** -0.5

# Set by test harness: trace=True to collect an NTFF profile.
TRACE = False
LAST_RESULTS = None


def _build():
    nc = bacc.Bacc("TRN2", target_bir_lowering=False, debug=False,
                   num_devices=NCORES)

    t = nc.dram_tensor("t", [C, NQ], BF16, kind="ExternalInput")
    r = nc.dram_tensor("r", [C, N], BF16, kind="ExternalInput")
    m = nc.dram_tensor("m", [C, C], BF16, kind="ExternalInput")
    wvT = nc.dram_tensor("wvT", [C, C], BF16, kind="ExternalInput")
    g = nc.dram_tensor("g", [C, 1], F32, kind="ExternalInput")
    o = nc.dram_tensor("o", [C, NQ], F32, kind="ExternalOutput")
    e_out = nc.dram_tensor("e_out", [N, NQ], BF16, kind="ExternalOutput")

    with tile.TileContext(nc) as tc:
        with (
            tc.tile_pool(name="persist", bufs=1) as persist,
            tc.tile_pool(name="epool", bufs=4) as epool,
            tc.tile_pool(name="outp", bufs=4) as outp,
            tc.tile_pool(name="ps_s", bufs=2, space="PSUM") as ps_s,
            tc.tile_pool(name="ps_av", bufs=4, space="PSUM") as ps_av,
        ):
            # ---- load inputs: constants first, then t (which gates the
            # u-projection and thereby everything), then r chunk by chunk.
            t_sb, r_sb, m_sb, wv_sb, g_sb = [], [], [], [], []
            for cc in range(CB):
                cs = slice(cc * P, (cc + 1) * P)
                for lst, src, nm in ((m_sb, m, "m"), (wv_sb, wvT, "wv")):
                    wtile = persist.tile([P, C], BF16, tag=f"{nm}{cc}")
                    nc.sync.dma_start(out=wtile[:], in_=src[cs, :])
                    lst.append(wtile)
                gt = persist.tile([P, 1], F32, tag=f"g{cc}")
                nc.sync.dma_start(out=gt[:], in_=g[cs, :])
                g_sb.append(gt)
                r_sb.append([persist.tile([P, RCH], BF16, tag=f"r{cc}_{ch}",
                                          name=f"r{cc}_{ch}")
                             for ch in range(N // RCH)])
                t_sb.append([persist.tile([P, RCH], BF16, tag=f"t{cc}_{ch}",
                                          name=f"t{cc}_{ch}")
                             for ch in range(NQ // RCH)])
            for ch in range(NQ // RCH):
                for cc in range(CB):
                    cs = slice(cc * P, (cc + 1) * P)
                    nc.sync.dma_start(out=t_sb[cc][ch][:],
                                      in_=t[cs, ch * RCH:(ch + 1) * RCH])
            for ch in range(N // RCH):
                for cc in range(CB):
                    cs = slice(cc * P, (cc + 1) * P)
                    nc.sync.dma_start(out=r_sb[cc][ch][:],
                                      in_=r[cs, ch * RCH:(ch + 1) * RCH])

            def r_slice(cc, jb):
                return r_sb[cc][(jb * P) // RCH][
                    :, (jb * P) % RCH:(jb * P) % RCH + P]

            # ---- projections ------------------------------------------------
            # u[b, i] = sum_a m[a, b] t[a, i]  (+g on the DVE copy)
            u_sb = [persist.tile([P, NQ], BF16, tag=f"u{bb}", name=f"u{bb}")
                    for bb in range(CB)]
            for half in range(NQ // RCH):
                for bb in range(CB):
                    bs = slice(bb * P, (bb + 1) * P)
                    up = ps_s.tile([P, RCH], F32, tag="s", name="up")
                    for ac in range(CB):
                        for nch in range(2):
                            nc.tensor.matmul(
                                up[:, nch * 512:(nch + 1) * 512],
                                lhsT=m_sb[ac][:, bs],
                                rhs=t_sb[ac][half][:, nch * 512:
                                                   (nch + 1) * 512],
                                start=(ac == 0), stop=(ac == CB - 1),
                            )
                    nc.vector.tensor_scalar_add(
                        u_sb[bb][:, half * RCH:(half + 1) * RCH], up[:],
                        g_sb[bb][:])

            # vT[j, c] = sum_c' r[c', j] wvT[c', c]  (r is the stationary op);
            # two key blocks share one PSUM tile and one DVE copy.
            v_sb = persist.tile([P, NJB * C], BF16, tag="v")
            for jp in range(NJB // 2):
                vp = ps_av.tile([P, 2 * C], F32, tag="av", name="vp")
                for j2 in range(2):
                    jb = 2 * jp + j2
                    for cc in range(CB):
                        nc.tensor.matmul(
                            vp[:, j2 * C:(j2 + 1) * C],
                            lhsT=r_slice(cc, jb),
                            rhs=wv_sb[cc][:],
                            start=(cc == 0), stop=(cc == CB - 1),
                        )
                nc.vector.tensor_copy(out=v_sb[:, jp * 2 * C:(jp + 1) * 2 * C],
                                      in_=vp[:])

            # ---- attention: pairs of query chunks ---------------------------
            for icp in range(NICH // 2):
                av = [ps_av.tile([P, ICH], F32, tag="av", name=f"av{icp}_{k}")
                      for k in range(2 * CB)]  # index = cb * 2 + ic2
                ets = {}

                def emit_scores(jb, icp=icp, ets=ets):
                    sps = ps_s.tile([P, 2 * ICH], F32, tag="s", name="sps")
                    for cc in range(CB):
                        for ic2 in range(2):
                            isl = slice((2 * icp + ic2) * ICH,
                                        (2 * icp + ic2 + 1) * ICH)
                            nc.tensor.matmul(
                                sps[:, ic2 * ICH:(ic2 + 1) * ICH],
                                lhsT=r_slice(cc, jb),
                                rhs=u_sb[cc][:, isl],
                                start=(cc == 0), stop=(cc == CB - 1),
                            )
                    et = epool.tile([P, 2 * ICH], BF16, tag="e", name="et")
                    nc.scalar.activation(et[:], sps[:],
                                         mybir.ActivationFunctionType.Exp,
                                         scale=SCALE)
                    nc.sync.dma_start(
                        out=e_out[jb * P:(jb + 1) * P,
                                  icp * 2 * ICH:(icp + 1) * 2 * ICH],
                        in_=et[:])
                    ets[jb] = et

                def emit_av(jb, icp=icp, av=av, ets=ets, final=False):
                    et = ets.pop(jb)
                    for cb in range(CB):
                        for ic2 in range(2):
                            k = cb * 2 + ic2
                            nc.tensor.matmul(
                                av[k][:],
                                lhsT=v_sb[:, jb * C + cb * P:
                                          jb * C + (cb + 1) * P],
                                rhs=et[:, ic2 * ICH:(ic2 + 1) * ICH],
                                start=(jb == 0), stop=(jb == NJB - 1),
                            )
                            if final:
                                # evacuate PSUM right behind the last matmul,
                                # alternating engines so the 4 copies drain in
                                # ~2 copy-times and free the bank for the next
                                # query-chunk pair.
                                isl = slice((2 * icp + ic2) * ICH,
                                            (2 * icp + ic2 + 1) * ICH)
                                ot = outp.tile([P, ICH], F32, tag="o",
                                               name="ot")
                                if k % 2 == 0:
                                    nc.vector.tensor_copy(out=ot[:],
                                                          in_=av[k][:])
                                else:
                                    nc.scalar.copy(ot[:], av[k][:])
                                nc.sync.dma_start(
                                    out=o[cb * P:(cb + 1) * P, isl],
                                    in_=ot[:])

                emit_scores(0)
                for jb in range(1, NJB):
                    emit_scores(jb)
                    emit_av(jb - 1)
                emit_av(NJB - 1, final=True)

    nc.finalize()
    return nc


_NC_CACHE = None


def kernel(target, reference, Wq, bq, Wk, bk, Wv, bv):
    global _NC_CACHE, LAST_RESULTS
    target = np.asarray(target, np.float32)
    reference = np.asarray(reference, np.float32)
    Wq, Wk, Wv = (np.asarray(w, np.float32) for w in (Wq, Wk, Wv))
    bq, bk, bv = (np.asarray(b_, np.float32) for b_ in (bq, bk, bv))

    if _NC_CACHE is None:
        _NC_CACHE = _build()
    nc = _NC_CACHE

    t_full = target.reshape(B, C, N)
    r_full = reference.reshape(B, C, N)
    m_mat = (Wq.T @ Wk).astype(NPBF16)           # scores fold: M = Wq^T Wk
    g_vec = (Wk.T @ bq).reshape(C, 1)            # bq fold (bk cancels exactly)
    w_common = {
        "m": m_mat,
        "wvT": np.ascontiguousarray(Wv.T).astype(NPBF16),
        "g": g_vec,
    }
    in_maps = []
    for cid in range(NCORES):
        b_, h_ = cid // 2, cid % 2
        in_maps.append({
            "t": np.ascontiguousarray(
                t_full[b_][:, h_ * NQ:(h_ + 1) * NQ]).astype(NPBF16),
            "r": r_full[b_].astype(NPBF16),
            **w_common,
        })

    res = run_bass_kernel_spmd(
        nc, in_maps, core_ids=list(range(NCORES)), trace=TRACE,
    )
    LAST_RESULTS = res

    out = np.empty((B, C, N), np.float32)
    for cid in range(NCORES):
        b_, h_ = cid // 2, cid % 2
        o = res.results[cid]["o"].astype(np.float64)
        den = res.results[cid]["e_out"].astype(np.float32).sum(
            axis=0, dtype=np.float64)
        sl = slice(h_ * NQ, (h_ + 1) * NQ)
        out[b_][:, sl] = (o / den[None, :] + bv.astype(np.float64)[:, None]
                          + t_full[b_][:, sl])
    return out.reshape(B, C, H, W)


# revision 22
# speedup vs baseline: 1.8724x; 1.2147x over previous
"""Cross-attention (B=4, C=256, H=W=64) Trainium2 Bass kernel.

Math (per batch b), with t = target[b] : [C, N], r = reference[b], N = H*W:
    q = Wq t + bq ; k = Wk r + bk ; v = Wv r + bv
    attn = softmax(q^T k / sqrt(C), axis=j)
    out = v attn^T + t

Sharding: 8 cores = 4 batches x 2 query-halves. Each core handles its
query slice of t (NQ = 2048) and the full r of its batch.

Algebraic folds (all exact):
  * scores: q_i . k_j = t_i^T (Wq^T Wk) r_j + bq.(Wk r_j) + (Wq t_i).bk + bq.bk
    The last two terms are per-query constants -> cancel in softmax.
    So with M = Wq^T Wk and g = Wk^T bq:  s[i,j] ~ r_j . u_i  where
    u = M^T t + g.  M, g are precomputed on the host.
  * bv: softmax rows sum to 1, so v -> v + bv just adds bv to the output;
    the host adds it.
  * normalization: the device returns o[c,i] = sum_j v[c,j] exp(s_ij)
    and the bf16 exp-matrix E; the host divides by colsum(E) (the exact
    denominator the AV matmul consumed) and adds the residual.

Device layouts (matmuls contract over the partition axis):
    u_sb[bb] : [128, NQ]  bf16   scores rhs
    r_sb[cc][ch] : [128, 1024] bf16 x4   scores + v-proj stationary operand
                  (chunked tiles so compute starts as soon as data lands)
    v_sb     : [128, NJB*C] bf16  V^T per key block: [j in block, c]
    scores   : S^T[j_blk, (ic2, i)] in a [128, 1024] PSUM tile; one exp
               (ACT) per key block covering a PAIR of query chunks, so
               each stationary tile serves 2 back-to-back matmuls; the
               AV pass runs one key block behind so exp latency hides.
"""

import os
import sys

import numpy as np

try:
    import concourse.bass as _probe  # noqa: F401
except ImportError:
    for _p in ("/opt/trn_rl_repo", "/root/.axon_site/_ro/trn_rl_repo"):
        if os.path.isdir(_p) and _p not in sys.path:
            sys.path.insert(0, _p)

import ml_dtypes

import concourse.bacc as bacc
import concourse.mybir as mybir
import concourse.tile as tile
from concourse.bass_utils import run_bass_kernel_spmd

BF16 = mybir.dt.bfloat16
FP8 = mybir.dt.float8e4
F32 = mybir.dt.float32
NPBF16 = ml_dtypes.bfloat16

B, C, H, W = 4, 256, 64, 64
N = H * W                 # 4096 key/value pixels per batch
NCORES = 8
NQ = (B * N) // NCORES    # 2048 query pixels per core
P = 128
CB = C // P               # 2 channel blocks
ICH = 512                 # query chunk (one PSUM bank of fp32)
NICH = NQ // ICH          # 4
NJB = N // P              # 32 key blocks
RCH = 1024                # r/t chunk width (per-chunk SBUF tiles)
SCALE = float(C) ** -0.5
EXP_BIAS = float(np.log(1 / 32.0))  # fp8e4m3 headroom (max finite 240, seen
                                    # scores reach ~7.9); the factor cancels
                                    # exactly in the numerator/denominator

# Set by test harness: trace=True to collect an NTFF profile.
TRACE = False
LAST_RESULTS = None


def _build():
    nc = bacc.Bacc("TRN2", target_bir_lowering=False, debug=False,
                   num_devices=NCORES)

    t = nc.dram_tensor("t", [C, NQ], BF16, kind="ExternalInput")
    r = nc.dram_tensor("r", [C, N], BF16, kind="ExternalInput")
    m = nc.dram_tensor("m", [C, C], BF16, kind="ExternalInput")
    wvT = nc.dram_tensor("wvT", [C, C], BF16, kind="ExternalInput")
    g = nc.dram_tensor("g", [C, 1], F32, kind="ExternalInput")
    o = nc.dram_tensor("o", [C, NQ], F32, kind="ExternalOutput")
    e_out = nc.dram_tensor("e_out", [N // 2, 2 * NQ], FP8, kind="ExternalOutput")

    with tile.TileContext(nc) as tc:
        with (
            tc.tile_pool(name="persist", bufs=1) as persist,
            tc.tile_pool(name="epool", bufs=4) as epool,
            tc.tile_pool(name="outp", bufs=4) as outp,
            tc.tile_pool(name="ps_s", bufs=2, space="PSUM") as ps_s,
            tc.tile_pool(name="ps_av", bufs=4, space="PSUM") as ps_av,
        ):
            # ---- load inputs: constants first, then t (which gates the
            # u-projection and thereby everything), then r chunk by chunk.
            t_sb, r_sb, m_sb, wv_sb, g_sb = [], [], [], [], []
            for cc in range(CB):
                cs = slice(cc * P, (cc + 1) * P)
                for lst, src, nm in ((m_sb, m, "m"), (wv_sb, wvT, "wv")):
                    wtile = persist.tile([P, C], BF16, tag=f"{nm}{cc}")
                    nc.sync.dma_start(out=wtile[:], in_=src[cs, :])
                    lst.append(wtile)
                gt = persist.tile([P, 1], F32, tag=f"g{cc}")
                nc.sync.dma_start(out=gt[:], in_=g[cs, :])
                g_sb.append(gt)
                r_sb.append([persist.tile([P, RCH], BF16, tag=f"r{cc}_{ch}",
                                          name=f"r{cc}_{ch}")
                             for ch in range(N // RCH)])
                t_sb.append([persist.tile([P, RCH], BF16, tag=f"t{cc}_{ch}",
                                          name=f"t{cc}_{ch}")
                             for ch in range(NQ // RCH)])
            for ch in range(NQ // RCH):
                for cc in range(CB):
                    cs = slice(cc * P, (cc + 1) * P)
                    nc.sync.dma_start(out=t_sb[cc][ch][:],
                                      in_=t[cs, ch * RCH:(ch + 1) * RCH])
            for ch in range(N // RCH):
                for cc in range(CB):
                    cs = slice(cc * P, (cc + 1) * P)
                    nc.sync.dma_start(out=r_sb[cc][ch][:],
                                      in_=r[cs, ch * RCH:(ch + 1) * RCH])

            exp_bias = persist.tile([P, 1], F32, tag="expbias")
            nc.vector.memset(exp_bias[:], EXP_BIAS)

            def r_slice(cc, jb):
                return r_sb[cc][(jb * P) // RCH][
                    :, (jb * P) % RCH:(jb * P) % RCH + P]

            # ---- projections ------------------------------------------------
            # u[b, i] = sum_a m[a, b] t[a, i]  (+g on the DVE copy)
            u_sb = [persist.tile([P, NQ], BF16, tag=f"u{bb}", name=f"u{bb}")
                    for bb in range(CB)]
            for half in range(NQ // RCH):
                for bb in range(CB):
                    bs = slice(bb * P, (bb + 1) * P)
                    up = ps_s.tile([P, RCH], F32, tag="s", name="up")
                    for ac in range(CB):
                        for nch in range(2):
                            nc.tensor.matmul(
                                up[:, nch * 512:(nch + 1) * 512],
                                lhsT=m_sb[ac][:, bs],
                                rhs=t_sb[ac][half][:, nch * 512:
                                                   (nch + 1) * 512],
                                start=(ac == 0), stop=(ac == CB - 1),
                            )
                    nc.vector.tensor_scalar_add(
                        u_sb[bb][:, half * RCH:(half + 1) * RCH], up[:],
                        g_sb[bb][:])

            # vT[j, c] = sum_c' r[c', j] wvT[c', c]  (r is the stationary op);
            # two key blocks share one PSUM tile and one DVE copy. Stored
            # fp8 in (j_hi, c)-per-pair layout, ready as DoubleRow weights.
            v_sb = persist.tile([P, NJB * C], FP8, tag="v")
            for jp in range(NJB // 2):
                vp = ps_av.tile([P, 2 * C], F32, tag="av", name="vp")
                for j2 in range(2):
                    jb = 2 * jp + j2
                    for cc in range(CB):
                        nc.tensor.matmul(
                            vp[:, j2 * C:(j2 + 1) * C],
                            lhsT=r_slice(cc, jb),
                            rhs=wv_sb[cc][:],
                            start=(cc == 0), stop=(cc == CB - 1),
                        )
                nc.vector.tensor_copy(out=v_sb[:, jp * 2 * C:(jp + 1) * 2 * C],
                                      in_=vp[:])

            # ---- attention: pairs of query chunks ---------------------------
            # exp writes fp8 E into per-key-pair tiles [128, (j_hi, ic2, i)];
            # the AV pass consumes a 256-wide contraction per DoubleRow
            # matmul, running a pair behind the score pass so exp hides.
            NJ2 = NJB // 2
            for icp in range(NICH // 2):
                av = [ps_av.tile([P, ICH], F32, tag="av", name=f"av{icp}_{k}")
                      for k in range(2 * CB)]  # index = cb * 2 + ic2
                ets = {}

                def emit_scores(jb, icp=icp, ets=ets):
                    jpair, jhi = jb // 2, jb % 2
                    sps = ps_s.tile([P, 2 * ICH], F32, tag="s", name="sps")
                    for cc in range(CB):
                        for ic2 in range(2):
                            isl = slice((2 * icp + ic2) * ICH,
                                        (2 * icp + ic2 + 1) * ICH)
                            nc.tensor.matmul(
                                sps[:, ic2 * ICH:(ic2 + 1) * ICH],
                                lhsT=r_slice(cc, jb),
                                rhs=u_sb[cc][:, isl],
                                start=(cc == 0), stop=(cc == CB - 1),
                            )
                    if jhi == 0:
                        ets[jpair] = epool.tile([P, 4 * ICH], FP8, tag="e",
                                                name="et")
                    et = ets[jpair]
                    nc.scalar.activation(et[:, jhi * 2 * ICH:
                                            (jhi + 1) * 2 * ICH], sps[:],
                                         mybir.ActivationFunctionType.Exp,
                                         scale=SCALE, bias=exp_bias[:])
                    if jhi == 1:
                        nc.sync.dma_start(
                            out=e_out[jpair * P:(jpair + 1) * P,
                                      icp * 4 * ICH:(icp + 1) * 4 * ICH],
                            in_=et[:])

                def emit_av(jpair, icp=icp, av=av, ets=ets, final=False):
                    et = ets.pop(jpair)
                    et3 = et.rearrange("p (h x) -> p h x", h=2)
                    for cb in range(CB):
                        v_ap = v_sb[:, jpair * 2 * C:(jpair + 1) * 2 * C
                                    ].rearrange("p (h c) -> p h c", h=2
                                                )[:, :, cb * P:(cb + 1) * P]
                        for ic2 in range(2):
                            k = cb * 2 + ic2
                            nc.tensor.matmul(
                                av[k][:],
                                lhsT=v_ap,
                                rhs=et3[:, :, ic2 * ICH:(ic2 + 1) * ICH],
                                start=(jpair == 0), stop=(jpair == NJ2 - 1),
                                perf_mode=mybir.MatmulPerfMode.DoubleRow,
                            )
                            if final:
                                # evacuate PSUM right behind the last matmul,
                                # alternating engines so the 4 copies drain in
                                # ~2 copy-times and free the bank for the next
                                # query-chunk pair.
                                isl = slice((2 * icp + ic2) * ICH,
                                            (2 * icp + ic2 + 1) * ICH)
                                ot = outp.tile([P, ICH], F32, tag="o",
                                               name="ot")
                                if k % 2 == 0:
                                    nc.vector.tensor_copy(out=ot[:],
                                                          in_=av[k][:])
                                else:
                                    nc.scalar.copy(ot[:], av[k][:])
                                nc.sync.dma_start(
                                    out=o[cb * P:(cb + 1) * P, isl],
                                    in_=ot[:])

                emit_scores(0)
                emit_scores(1)
                for jpair in range(1, NJ2):
                    emit_scores(2 * jpair)
                    emit_scores(2 * jpair + 1)
                    emit_av(jpair - 1)
                emit_av(NJ2 - 1, final=True)

    nc.finalize()
    return nc


_NC_CACHE = None


def kernel(target, reference, Wq, bq, Wk, bk, Wv, bv):
    global _NC_CACHE, LAST_RESULTS
    target = np.asarray(target, np.float32)
    reference = np.asarray(reference, np.float32)
    Wq, Wk, Wv = (np.asarray(w, np.float32) for w in (Wq, Wk, Wv))
    bq, bk, bv = (np.asarray(b_, np.float32) for b_ in (bq, bk, bv))

    if _NC_CACHE is None:
        _NC_CACHE = _build()
    nc = _NC_CACHE

    t_full = target.reshape(B, C, N)
    r_full = reference.reshape(B, C, N)
    m_mat = (Wq.T @ Wk).astype(NPBF16)           # scores fold: M = Wq^T Wk
    g_vec = (Wk.T @ bq).reshape(C, 1)            # bq fold (bk cancels exactly)
    w_common = {
        "m": m_mat,
        "wvT": np.ascontiguousarray(Wv.T).astype(NPBF16),
        "g": g_vec,
    }
    in_maps = []
    for cid in range(NCORES):
        b_, h_ = cid // 2, cid % 2
        in_maps.append({
            "t": np.ascontiguousarray(
                t_full[b_][:, h_ * NQ:(h_ + 1) * NQ]).astype(NPBF16),
            "r": r_full[b_].astype(NPBF16),
            **w_common,
        })

    res = run_bass_kernel_spmd(
        nc, in_maps, core_ids=list(range(NCORES)), trace=TRACE,
    )
    LAST_RESULTS = res

    out = np.empty((B, C, N), np.float32)
    for cid in range(NCORES):
        b_, h_ = cid // 2, cid % 2
        o = res.results[cid]["o"].astype(np.float64)
        # e_out cols per icp-block: (j_hi, ic2, i); denominator sums the
        # exact fp8 values the AV matmul consumed.
        e = res.results[cid]["e_out"].astype(np.float32)
        den = e.reshape(N // 2, NICH // 2, 2, NQ // 2).sum(
            axis=(0, 2), dtype=np.float64).reshape(NQ)
        sl = slice(h_ * NQ, (h_ + 1) * NQ)
        out[b_][:, sl] = (o / den[None, :] + bv.astype(np.float64)[:, None]
                          + t_full[b_][:, sl])
    return out.reshape(B, C, H, W)
